# revision 8
# baseline (speedup 1.0000x reference)
"""Multi-head self-attention (causal) Trainium2 kernel, 8-way sharded.

Sharding: core c handles batch b = c//4 and head group g = c%4 (4 of 16
heads). Each core computes q/k/v projections for its head slice, causal
softmax attention, and a partial o_proj ([2048, 1024] bf16); the host
sums the 4 partials per batch in f32.

Layouts (per core):
  xT    [1024, 2048]  x[b].T            (d_model on partitions)
  wqT   [1024,  256]  Wq[g*256:(g+1)*256, :].T      (same for wk/wv)
  woT   [ 256, 1024]  Wo[:, g*256:(g+1)*256].T
  utri  [ 128,  128]  -200 where p > j (strict lower = masked keys)
  ident [ 128,  128]  identity (stationary for the causal-bias matmul)

Everything on-chip is bf16 (PSUM accumulation stays fp32): bf16 matmuls
stream 1 col/cycle at any width (f32r pays 2-4x below 256 cols), so the
diagonal chunks run at exact 128-col granularity with a single 128-wide
causal-bias matmul (ident.T @ utri = -200 above the diagonal; exp -> 0).

q/k projections run with 4 PSUM groups per slab so matmuls start as soon
as the first x chunk lands. kT/qT are stored head-major stacked two
heads per partition column; S matmuls contract over K=64 partition
slices so no zero padding is needed.

Startup: the first matmuls need only wq + x slab0, so those four DMAs
issue first on four separate queues (sync/scalar HWDGE + gpsimd/vector);
everything else queues behind. Attention pipeline runs at depth 3
(S(kt) issues, O(kt-2) pops) so the ACT exp latency (~1.15us per kt
chunk) stays off the PE critical path. V carries an appended ones column
so PSUM partition 64 accumulates the softmax sums; normalization is a
custom-DVE reciprocal_approx_fast + GpSimd partition_broadcast + DVE
multiply. o_proj chunks are [128, 1024] with a single out DMA each.
"""

import ml_dtypes
import numpy as np

import concourse.bass as bass
import concourse.mybir as mybir
import concourse.tile as tile
from concourse import bacc
from concourse.bass_utils import run_bass_kernel_spmd

P = 128
S = 2048  # sequence length
DM = 1024  # d_model
HD = 64  # head dim
NH_CORE = 4  # heads per core
HSL = NH_CORE * HD  # head slice width = 256
QC = 512  # query chunk
N_QC = S // QC  # 4
N_KT = S // P  # 16 key tiles
KO = DM // P  # 8 k-tiles over d_model

f32 = mybir.dt.float32
bf16 = mybir.dt.bfloat16

_CACHED = {}


def build_program():
    nc = bacc.Bacc("TRN2", target_bir_lowering=False, debug=False)
    # all inputs host-prearranged into SBUF tile layouts so every DMA line
    # is one long contiguous read per partition (no strided descriptors)
    xS = nc.declare_dram_parameter("xS", [N_QC, P, KO, QC], bf16, isOutput=False)
    wqT = nc.declare_dram_parameter("wqT", [P, KO, HSL], bf16, isOutput=False)
    wkT = nc.declare_dram_parameter("wkT", [P, KO, HSL], bf16, isOutput=False)
    wvT = nc.declare_dram_parameter("wvT", [P, KO, HSL], bf16, isOutput=False)
    woT = nc.declare_dram_parameter("woT", [P, 2, DM], bf16, isOutput=False)
    utri = nc.declare_dram_parameter("utri", [P, P], bf16, isOutput=False)
    ident = nc.declare_dram_parameter("ident", [P, P], bf16, isOutput=False)
    out = nc.declare_dram_parameter("out", [S, DM], bf16, isOutput=True)

    with tile.TileContext(nc) as tc:
        with (
            tc.tile_pool(name="persist", bufs=1) as persist,
            tc.tile_pool(name="small", bufs=2) as small,
        ):
            # ---- persistent tiles
            qTr = persist.tile([P, NH_CORE, S], bf16, tag="qTr")
            kTr = persist.tile([P, 2, S], bf16, tag="kTr")
            vr = persist.tile([P, N_KT, NH_CORE, HD + 1], bf16, tag="vr")
            woTr = persist.tile([P, 2, DM], bf16, tag="woTr")
            aTr = persist.tile([P, 2, S], bf16, tag="aTr")
            utri_sb = persist.tile([P, P], bf16, tag="utri")
            ident_sb = persist.tile([P, P], bf16, tag="ident")

            # ---- phase 0+1: load x/weights, projections.
            with tc.tile_pool(name="xw", bufs=1) as xw:
                # slab-major so each slab DMA writes one contiguous
                # 16KB-per-partition block (hardware-dynamic descriptors)
                xTr = xw.tile([P, N_QC, KO, QC], bf16, tag="xTr")
                wts = {}
                for name, dram in (("q", wqT), ("k", wkT), ("v", wvT)):
                    wts[name] = xw.tile(
                        [P, KO, HSL], bf16, tag=f"w{name}r", name=f"w{name}r"
                    )
                # critical path first: the four pieces the first projection
                # matmuls need go out on four parallel queues; the rest
                # queues behind in consumption order.
                nc.sync.dma_start(wts["q"][:, 0:2, :], wqT[:, 0:2, :])
                nc.gpsimd.dma_start(xTr[:, 0, 0:1, :], xS[0, :, 0:1, :])
                nc.scalar.dma_start(xTr[:, 0, 1:2, :], xS[0, :, 1:2, :])
                nc.sync.dma_start(wts["q"][:, 2:4, :], wqT[:, 2:4, :])
                nc.gpsimd.dma_start(xTr[:, 0, 2:3, :], xS[0, :, 2:3, :])
                nc.scalar.dma_start(xTr[:, 0, 3:4, :], xS[0, :, 3:4, :])
                nc.sync.dma_start(wts["q"][:, 4:8, :], wqT[:, 4:8, :])
                nc.gpsimd.dma_start(xTr[:, 0, 4:6, :], xS[0, :, 4:6, :])
                nc.scalar.dma_start(xTr[:, 0, 6:8, :], xS[0, :, 6:8, :])
                nc.sync.dma_start(wts["k"][:, 0:4, :], wkT[:, 0:4, :])
                nc.scalar.dma_start(wts["k"][:, 4:8, :], wkT[:, 4:8, :])
                nc.gpsimd.dma_start(xTr[:, 1], xS[1])
                nc.sync.dma_start(wts["v"][:, 0:4, :], wvT[:, 0:4, :])
                nc.scalar.dma_start(wts["v"][:, 4:8, :], wvT[:, 4:8, :])
                nc.gpsimd.dma_start(xTr[:, 2], xS[2])
                nc.scalar.dma_start(woTr[:], woT[:])
                nc.gpsimd.dma_start(xTr[:, 3], xS[3])
                nc.sync.dma_start(utri_sb[:], utri[:])
                nc.sync.dma_start(ident_sb[:], ident[:])
                nc.vector.memset(
                    vr[:, :, :, HD].rearrange("p a b -> p (a b)"), 1.0
                )
                # zero the pad halves of qTr: even heads live on partitions
                # 0:64, odd heads on 64:128 (the other half multiplies the
                # co-resident head's k rows, so it must be zero)
                zeros_f = persist.tile([P, 1], bf16, tag="zeros")
                nc.vector.memset(zeros_f[:], 0.0)
                nc.vector.tensor_copy(
                    qTr[HD:P, 0::2, :],
                    zeros_f[HD:P, 0:1, None].to_broadcast([HD, 2, S]),
                )
                nc.vector.tensor_copy(
                    qTr[0:HD, 1::2, :],
                    zeros_f[0:HD, 0:1, None].to_broadcast([HD, 2, S]),
                )

                with (
                    tc.tile_pool(name="ps_qk", bufs=1, space="PSUM") as ps_qk,
                    tc.tile_pool(name="ps_v", bufs=4, space="PSUM") as ps_v,
                ):
                    for sl in range(N_QC):
                        ssl = slice(sl * QC, (sl + 1) * QC)
                        for name, mt in (
                            ("q", 0), ("q", 1), ("k", 0), ("k", 1),
                        ):
                            wr = wts[name]
                            ps = ps_qk.tile(
                                [P, QC], f32,
                                tag=f"pg{name}{mt}", name=f"pg{name}{mt}",
                            )
                            for ko in range(KO):
                                nc.tensor.matmul(
                                    ps[:],
                                    wr[:, ko, mt * P : (mt + 1) * P],
                                    xTr[:, sl, ko, :],
                                    start=(ko == 0),
                                    stop=(ko == KO - 1),
                                )
                            if name == "k":
                                nc.vector.tensor_copy(
                                    kTr[:, mt, ssl], ps[:]
                                )
                            else:
                                nc.vector.tensor_copy(
                                    qTr[0:HD, 2 * mt, ssl], ps[0:HD, :]
                                )
                                nc.vector.tensor_copy(
                                    qTr[HD:P, 2 * mt + 1, ssl], ps[HD:P, :]
                                )
                        # v seq tiles of this slab
                        wr = wts["v"]
                        for st4 in range(4):
                            st = 4 * sl + st4
                            ps = ps_v.tile([P, HSL], f32, tag="pv")
                            for ko in range(KO):
                                nc.tensor.matmul(
                                    ps[:],
                                    xTr[:, sl, ko, st4 * P : (st4 + 1) * P],
                                    wr[:, ko, :],
                                    start=(ko == 0),
                                    stop=(ko == KO - 1),
                                )
                            nc.scalar.activation(
                                vr[:, st, :, 0:HD],
                                ps[:].rearrange("p (h d) -> p h d", d=HD),
                                mybir.ActivationFunctionType.Copy,
                            )

            # ---- phase 2: attention per (query-pair, head), with the
            # o_proj for pr0's query tiles interleaved into pr1's PE stream
            with (
                tc.tile_pool(name="expr", bufs=4) as expr,
                tc.tile_pool(name="ps_s", bufs=2, space="PSUM") as ps_s,
                tc.tile_pool(name="ps_ot", bufs=1, space="PSUM") as ps_ot,
                tc.tile_pool(name="ps_o", bufs=2, space="PSUM") as ps_o,
                tc.tile_pool(name="outp", bufs=3) as outp,
            ):

                def normalize(h, qc, ps_acc):
                    # sums row copied to partition 0 first (the custom-DVE
                    # reciprocal produces garbage on partition-offset
                    # inputs); the final multiply reads the PSUM accumulator
                    # directly — the bank stays held ~2us longer but its
                    # next writer (same-tag O matmul, next head) is a full
                    # head away.
                    hm, hb = h // 2, (h % 2) * HD
                    sums = small.tile([1, QC], f32, tag="sums", name="sums")
                    nc.vector.tensor_copy(sums[:], ps_acc[HD : HD + 1, :])
                    recip = small.tile([1, QC], f32, tag="recip", name="recip")
                    nc.vector.reciprocal_approx_fast(recip[:], sums[:])
                    bcast = small.tile([HD, QC], f32, tag="bcast", name="bcast")
                    nc.gpsimd.partition_broadcast(bcast[:], recip[:])
                    nc.vector.tensor_mul(
                        aTr[hb : hb + HD, hm, qc * QC : (qc + 1) * QC],
                        ps_acc[0:HD, :],
                        bcast[:],
                    )

                def o_group(h, okt, segs, er_g, ps_ots):
                    for qc, c0, o0, w in reversed(segs):
                        nc.tensor.matmul(
                            ps_ots[qc][:, o0:QC],
                            vr[:, okt, h, :],
                            er_g[:, c0 : c0 + w],
                            start=(okt == 0),
                            stop=(okt == 4 * qc + 3),
                        )
                        if okt == 4 * qc + 3:
                            normalize(h, qc, ps_ots[qc])

                def o_chunk(st, engs):
                    # full o_proj row block for seq tile st: two PSUM halves,
                    # one [128, 1024] SBUF tile, one out DMA (2KB rows)
                    ot = outp.tile([P, DM], bf16, tag="ot", name="ot")
                    for nch in range(2):
                        ps = ps_o.tile([P, QC], f32, tag="po", name="po")
                        for kt2 in range(2):
                            nc.tensor.matmul(
                                ps[:],
                                aTr[:, kt2, st * P : (st + 1) * P],
                                woTr[:, kt2, nch * QC : (nch + 1) * QC],
                                start=(kt2 == 0),
                                stop=(kt2 == 1),
                            )
                        dst = ot[:, nch * QC : (nch + 1) * QC]
                        if engs[nch] == "v":
                            nc.vector.tensor_copy(dst, ps[:])
                        else:
                            nc.scalar.activation(
                                dst, ps[:], mybir.ActivationFunctionType.Copy
                            )
                    [nc.sync, nc.gpsimd, nc.scalar][st % 3].dma_start(
                        out[st * P : (st + 1) * P, :], ot[:]
                    )

                PEND = 3  # S->exp->O pipeline depth
                # flat (pr, h, kt) stream: the pipeline crosses head
                # boundaries, so the last exps of head h overlap the first
                # S matmuls of head h+1 (no per-head drain stall)
                pend = []

                def flush_one():
                    okt, oh, osegs, oer, ops_ots = pend.pop(0)
                    o_group(oh, okt, osegs, oer, ops_ots)

                for pr in range(2):
                    qcs = (2 * pr, 2 * pr + 1)
                    for h in range(NH_CORE):
                        hm = h // 2
                        if pr == 1:
                            # pr0's aTr is complete: slot its o_proj into
                            # this head's PE stream (fills the window while
                            # the previous head's accumulators normalize)
                            for st in (2 * h, 2 * h + 1):
                                o_chunk(st, ("v", "v"))
                        ps_ots = {
                            qc: ps_ot.tile(
                                [HD + 1, QC], f32,
                                tag=f"ot{qc % 2}", name="ps_ot",
                            )
                            for qc in qcs
                        }
                        for kt in range(8 * (pr + 1)):
                            jd = kt // 4
                            off = (kt % 4) * P
                            live = [qc for qc in qcs if qc >= jd]
                            ps_g = ps_s.tile(
                                [P, 2 * QC], f32, tag="ps_s", name="ps_g"
                            )
                            er_g = expr.tile(
                                [P, 2 * QC], bf16, tag="er", name="er_g"
                            )
                            segs = []
                            for qc in live:
                                diag = qc == jd
                                o0 = off if diag else 0
                                c0 = (qc - qcs[0]) * QC + o0
                                w = QC - o0
                                nc.tensor.matmul(
                                    ps_g[:, c0 : c0 + w],
                                    kTr[:, hm, kt * P : (kt + 1) * P],
                                    qTr[:, h, qc * QC + o0 : (qc + 1) * QC],
                                    start=True,
                                    stop=not diag,
                                )
                                if diag:
                                    # causal bias: += ident.T @ utri
                                    # (-200 above the diagonal; exp -> 0)
                                    nc.tensor.matmul(
                                        ps_g[:, c0 : c0 + P],
                                        ident_sb[:],
                                        utri_sb[:],
                                        start=False,
                                        stop=True,
                                    )
                                segs.append((qc, c0, o0, w))
                            g0 = segs[0][1]
                            g1 = segs[-1][1] + segs[-1][3]
                            nc.scalar.activation(
                                er_g[:, g0:g1],
                                ps_g[:, g0:g1],
                                mybir.ActivationFunctionType.Exp,
                                scale=0.125,
                            )
                            pend.append((kt, h, segs, er_g, ps_ots))
                            if len(pend) >= PEND:
                                flush_one()
                while pend:
                    flush_one()

                # ---- tail: o_proj for pr1's seq tiles (ACT is idle now,
                # so split the PSUM->SBUF copies between DVE and ACT)
                for st in range(8, N_KT):
                    o_chunk(st, ("v", "s") if st % 2 == 0 else ("s", "v"))

    nc.compile()
    return nc


def _make_utri():
    # utri[p, j] = -200 where p > j: biases the 128 diagonal query cols of
    # a diagonal key tile. After the 0.125 exp scale a masked logit sits at
    # <= -19 nats (exp <= 6e-9, negligible vs row sums >= 1) while staying
    # inside the ACT exp table's domain (huge negatives misbehave).
    p = np.arange(P)[:, None]
    j = np.arange(P)[None, :]
    return np.where(p > j, np.float32(-200.0), np.float32(0.0)).astype(
        ml_dtypes.bfloat16
    )


def make_in_maps(x, Wq, Wk, Wv, Wo):
    utri = _make_utri()
    ident = np.eye(P, dtype=np.float32).astype(ml_dtypes.bfloat16)

    def wtile(wT):  # [1024, 256] -> [128, 8, 256] (p, ko, m), bf16
        return np.ascontiguousarray(
            wT.reshape(KO, P, HSL).transpose(1, 0, 2)
        ).astype(ml_dtypes.bfloat16)

    in_maps = []
    for c in range(8):
        bi, g = c // 4, c % 4
        sl = slice(g * HSL, (g + 1) * HSL)
        # xS[sl, p, ko, s] = x[bi][sl*512+s, ko*128+p]
        xs = np.ascontiguousarray(
            x[bi].reshape(N_QC, QC, KO, P).transpose(0, 3, 2, 1)
        ).astype(ml_dtypes.bfloat16)
        in_maps.append(
            {
                "xS": xs,
                "wqT": wtile(Wq[sl, :].T),
                "wkT": wtile(Wk[sl, :].T),
                "wvT": wtile(Wv[sl, :].T),
                "woT": np.ascontiguousarray(
                    Wo[:, sl].T.reshape(2, P, DM).transpose(1, 0, 2)
                ).astype(ml_dtypes.bfloat16),
                "utri": utri,
                "ident": ident,
            }
        )
    return in_maps


def kernel(x, Wq, Wk, Wv, Wo):
    x = np.asarray(x, dtype=np.float32)
    Wq = np.asarray(Wq, dtype=np.float32)
    Wk = np.asarray(Wk, dtype=np.float32)
    Wv = np.asarray(Wv, dtype=np.float32)
    Wo = np.asarray(Wo, dtype=np.float32)
    b, s, dm = x.shape
    assert (b, s, dm) == (2, S, DM), (b, s, dm)

    if "nc" not in _CACHED:
        _CACHED["nc"] = build_program()
    nc = _CACHED["nc"]

    in_maps = make_in_maps(x, Wq, Wk, Wv, Wo)
    res = run_bass_kernel_spmd(nc, in_maps, core_ids=list(range(8)))

    out = np.zeros((2, S, DM), dtype=np.float32)
    for c in range(8):
        out[c // 4] += np.asarray(res.results[c]["out"]).astype(np.float32)
    return out


# revision 11
# speedup vs baseline: 1.0015x; 1.0015x over previous
"""Multi-head self-attention (causal) Trainium2 kernel, 8-way sharded.

Sharding: core c handles batch b = c//4 and head group g = c%4 (4 of 16
heads). Each core computes q/k/v projections for its head slice, causal
softmax attention, and a partial o_proj ([2048, 1024] bf16); the host
sums the 4 partials per batch in f32.

Layouts (per core):
  xT    [1024, 2048]  x[b].T            (d_model on partitions)
  wqT   [1024,  256]  Wq[g*256:(g+1)*256, :].T      (same for wk/wv)
  woT   [ 256, 1024]  Wo[:, g*256:(g+1)*256].T
  utri  [ 128,  128]  -200 where p > j (strict lower = masked keys)
  ident [ 128,  128]  identity (stationary for the causal-bias matmul)

Everything on-chip is bf16 (PSUM accumulation stays fp32): bf16 matmuls
stream 1 col/cycle at any width (f32r pays 2-4x below 256 cols), so the
diagonal chunks run at exact 128-col granularity with a single 128-wide
causal-bias matmul (ident.T @ utri = -200 above the diagonal; exp -> 0).

q/k projections run with 4 PSUM groups per slab so matmuls start as soon
as the first x chunk lands. kT/qT are stored head-major stacked two
heads per partition column; S matmuls contract over K=64 partition
slices so no zero padding is needed.

Startup: the first matmuls need only wq + x slab0, so those four DMAs
issue first on four separate queues (sync/scalar HWDGE + gpsimd/vector);
everything else queues behind. Attention pipeline runs at depth 3
(S(kt) issues, O(kt-2) pops) so the ACT exp latency (~1.15us per kt
chunk) stays off the PE critical path. V carries an appended ones column
so PSUM partition 64 accumulates the softmax sums; normalization is a
custom-DVE reciprocal_approx_fast + GpSimd partition_broadcast + DVE
multiply. o_proj chunks are [128, 1024] with a single out DMA each.
"""

import ml_dtypes
import numpy as np

import concourse.bass as bass
import concourse.mybir as mybir
import concourse.tile as tile
from concourse import bacc
from concourse.bass_utils import run_bass_kernel_spmd

P = 128
S = 2048  # sequence length
DM = 1024  # d_model
HD = 64  # head dim
NH_CORE = 4  # heads per core
HSL = NH_CORE * HD  # head slice width = 256
QC = 512  # query chunk
N_QC = S // QC  # 4
N_KT = S // P  # 16 key tiles
KO = DM // P  # 8 k-tiles over d_model

f32 = mybir.dt.float32
bf16 = mybir.dt.bfloat16

_CACHED = {}


def build_program():
    nc = bacc.Bacc("TRN2", target_bir_lowering=False, debug=False)
    # all inputs host-prearranged into SBUF tile layouts so every DMA line
    # is one long contiguous read per partition (no strided descriptors)
    xS = nc.declare_dram_parameter("xS", [N_QC, P, KO, QC], bf16, isOutput=False)
    wqT = nc.declare_dram_parameter("wqT", [P, KO, HSL], bf16, isOutput=False)
    wkT = nc.declare_dram_parameter("wkT", [P, KO, HSL], bf16, isOutput=False)
    wvT = nc.declare_dram_parameter("wvT", [P, KO, HSL], bf16, isOutput=False)
    woT = nc.declare_dram_parameter("woT", [P, 2, DM], bf16, isOutput=False)
    utri = nc.declare_dram_parameter("utri", [P, P], bf16, isOutput=False)
    ident = nc.declare_dram_parameter("ident", [P, P], bf16, isOutput=False)
    out = nc.declare_dram_parameter("out", [S, DM], bf16, isOutput=True)

    with tile.TileContext(nc) as tc:
        with (
            tc.tile_pool(name="persist", bufs=1) as persist,
            tc.tile_pool(name="small", bufs=2) as small,
        ):
            # ---- persistent tiles
            qTr = persist.tile([P, NH_CORE, S], bf16, tag="qTr")
            kTr = persist.tile([P, 2, S], bf16, tag="kTr")
            vr = persist.tile([P, N_KT, NH_CORE, HD + 1], bf16, tag="vr")
            woTr = persist.tile([P, 2, DM], bf16, tag="woTr")
            aTr = persist.tile([P, 2, S], bf16, tag="aTr")
            utri_sb = persist.tile([P, P], bf16, tag="utri")
            ident_sb = persist.tile([P, P], bf16, tag="ident")

            # ---- phase 0+1: load x/weights, projections.
            with tc.tile_pool(name="xw", bufs=1) as xw:
                # slab-major so each slab DMA writes one contiguous
                # 16KB-per-partition block (hardware-dynamic descriptors)
                xTr = xw.tile([P, N_QC, KO, QC], bf16, tag="xTr")
                wts = {}
                for name, dram in (("q", wqT), ("k", wkT), ("v", wvT)):
                    wts[name] = xw.tile(
                        [P, KO, HSL], bf16, tag=f"w{name}r", name=f"w{name}r"
                    )
                # critical path first: the four pieces the first projection
                # matmuls need go out on four parallel queues; the rest
                # queues behind in consumption order.
                # critical pieces (wq + slab0) partition-split with full
                # per-partition lines (4-8KB elements, descriptor-cheap)
                # across the three queues; everything else behind.
                nc.sync.dma_start(wts["q"][0:64], wqT[0:64])
                nc.scalar.dma_start(wts["q"][64:128], wqT[64:128])
                nc.gpsimd.dma_start(xTr[0:64, 0], xS[0, 0:64])
                nc.sync.dma_start(xTr[64:96, 0], xS[0, 64:96])
                nc.scalar.dma_start(xTr[96:128, 0], xS[0, 96:128])
                nc.sync.dma_start(wts["k"][0:64], wkT[0:64])
                nc.scalar.dma_start(wts["k"][64:128], wkT[64:128])
                nc.gpsimd.dma_start(xTr[:, 1], xS[1])
                nc.sync.dma_start(wts["v"][0:64], wvT[0:64])
                nc.scalar.dma_start(wts["v"][64:128], wvT[64:128])
                nc.gpsimd.dma_start(xTr[:, 2], xS[2])
                nc.scalar.dma_start(woTr[:], woT[:])
                nc.gpsimd.dma_start(xTr[:, 3], xS[3])
                nc.sync.dma_start(utri_sb[:], utri[:])
                nc.sync.dma_start(ident_sb[:], ident[:])
                nc.vector.memset(
                    vr[:, :, :, HD].rearrange("p a b -> p (a b)"), 1.0
                )
                # zero the pad halves of qTr: even heads live on partitions
                # 0:64, odd heads on 64:128 (the other half multiplies the
                # co-resident head's k rows, so it must be zero)
                zeros_f = persist.tile([P, 1], bf16, tag="zeros")
                nc.vector.memset(zeros_f[:], 0.0)
                nc.vector.tensor_copy(
                    qTr[HD:P, 0::2, :],
                    zeros_f[HD:P, 0:1, None].to_broadcast([HD, 2, S]),
                )
                nc.vector.tensor_copy(
                    qTr[0:HD, 1::2, :],
                    zeros_f[0:HD, 0:1, None].to_broadcast([HD, 2, S]),
                )

                with (
                    tc.tile_pool(name="ps_qk", bufs=1, space="PSUM") as ps_qk,
                    tc.tile_pool(name="ps_v", bufs=4, space="PSUM") as ps_v,
                ):
                    for sl in range(N_QC):
                        ssl = slice(sl * QC, (sl + 1) * QC)
                        for name, mt in (
                            ("q", 0), ("q", 1), ("k", 0), ("k", 1),
                        ):
                            wr = wts[name]
                            ps = ps_qk.tile(
                                [P, QC], f32,
                                tag=f"pg{name}{mt}", name=f"pg{name}{mt}",
                            )
                            for ko in range(KO):
                                nc.tensor.matmul(
                                    ps[:],
                                    wr[:, ko, mt * P : (mt + 1) * P],
                                    xTr[:, sl, ko, :],
                                    start=(ko == 0),
                                    stop=(ko == KO - 1),
                                )
                            if name == "k":
                                nc.vector.tensor_copy(
                                    kTr[:, mt, ssl], ps[:]
                                )
                            else:
                                nc.vector.tensor_copy(
                                    qTr[0:HD, 2 * mt, ssl], ps[0:HD, :]
                                )
                                nc.vector.tensor_copy(
                                    qTr[HD:P, 2 * mt + 1, ssl], ps[HD:P, :]
                                )
                        # v seq tiles of this slab
                        wr = wts["v"]
                        for st4 in range(4):
                            st = 4 * sl + st4
                            ps = ps_v.tile([P, HSL], f32, tag="pv")
                            for ko in range(KO):
                                nc.tensor.matmul(
                                    ps[:],
                                    xTr[:, sl, ko, st4 * P : (st4 + 1) * P],
                                    wr[:, ko, :],
                                    start=(ko == 0),
                                    stop=(ko == KO - 1),
                                )
                            nc.scalar.activation(
                                vr[:, st, :, 0:HD],
                                ps[:].rearrange("p (h d) -> p h d", d=HD),
                                mybir.ActivationFunctionType.Copy,
                            )

            # ---- phase 2: attention per (query-pair, head), with the
            # o_proj for pr0's query tiles interleaved into pr1's PE stream
            with (
                tc.tile_pool(name="expr", bufs=4) as expr,
                tc.tile_pool(name="ps_s", bufs=2, space="PSUM") as ps_s,
                tc.tile_pool(name="ps_ot", bufs=1, space="PSUM") as ps_ot,
                tc.tile_pool(name="ps_o", bufs=2, space="PSUM") as ps_o,
                tc.tile_pool(name="outp", bufs=3) as outp,
            ):

                def normalize(h, qc, ps_acc):
                    # copy the accumulator out first: releases the PSUM bank
                    # quickly (the next head's O matmuls reuse the tag ~3
                    # iterations later). The sums row is copied to partition
                    # 0 separately: the custom-DVE reciprocal produces
                    # garbage on partition-offset inputs.
                    hm, hb = h // 2, (h % 2) * HD
                    sums = small.tile([1, QC], f32, tag="sums", name="sums")
                    nc.vector.tensor_copy(sums[:], ps_acc[HD : HD + 1, :])
                    oc = small.tile([HD, QC], f32, tag="oc", name="oc")
                    nc.vector.tensor_copy(oc[:], ps_acc[0:HD, :])
                    recip = small.tile([1, QC], f32, tag="recip", name="recip")
                    nc.vector.reciprocal_approx_fast(recip[:], sums[:])
                    bcast = small.tile([HD, QC], f32, tag="bcast", name="bcast")
                    nc.gpsimd.partition_broadcast(bcast[:], recip[:])
                    nc.vector.tensor_mul(
                        aTr[hb : hb + HD, hm, qc * QC : (qc + 1) * QC],
                        oc[:],
                        bcast[:],
                    )

                def o_group(h, okt, segs, er_g, ps_ots):
                    for qc, c0, o0, w in reversed(segs):
                        nc.tensor.matmul(
                            ps_ots[qc][:, o0:QC],
                            vr[:, okt, h, :],
                            er_g[:, c0 : c0 + w],
                            start=(okt == 0),
                            stop=(okt == 4 * qc + 3),
                        )
                        if okt == 4 * qc + 3:
                            normalize(h, qc, ps_ots[qc])

                def o_chunk(st, engs):
                    # full o_proj row block for seq tile st: two PSUM halves,
                    # one [128, 1024] SBUF tile, one out DMA (2KB rows)
                    ot = outp.tile([P, DM], bf16, tag="ot", name="ot")
                    for nch in range(2):
                        ps = ps_o.tile([P, QC], f32, tag="po", name="po")
                        for kt2 in range(2):
                            nc.tensor.matmul(
                                ps[:],
                                aTr[:, kt2, st * P : (st + 1) * P],
                                woTr[:, kt2, nch * QC : (nch + 1) * QC],
                                start=(kt2 == 0),
                                stop=(kt2 == 1),
                            )
                        dst = ot[:, nch * QC : (nch + 1) * QC]
                        if engs[nch] == "v":
                            nc.vector.tensor_copy(dst, ps[:])
                        else:
                            nc.scalar.activation(
                                dst, ps[:], mybir.ActivationFunctionType.Copy
                            )
                    # out DMAs ride the two HWDGE queues, partition-split so
                    # the final chunks drain in parallel with 2KB rows
                    nc.sync.dma_start(
                        out[st * P : st * P + 64, :], ot[0:64, :]
                    )
                    nc.scalar.dma_start(
                        out[st * P + 64 : (st + 1) * P, :], ot[64:128, :]
                    )

                PEND = 3  # S->exp->O pipeline depth
                # flat (pr, h, kt) stream: the pipeline crosses head
                # boundaries, so the last exps of head h overlap the first
                # S matmuls of head h+1 (no per-head drain stall)
                pend = []

                def flush_one():
                    okt, oh, osegs, oer, ops_ots = pend.pop(0)
                    o_group(oh, okt, osegs, oer, ops_ots)

                for pr in range(2):
                    qcs = (2 * pr, 2 * pr + 1)
                    for h in range(NH_CORE):
                        hm = h // 2
                        if pr == 1:
                            # pr0's aTr is complete: slot its o_proj into
                            # this head's PE stream (fills the window while
                            # the previous head's accumulators normalize)
                            for st in (2 * h, 2 * h + 1):
                                o_chunk(st, ("v", "v"))
                        ps_ots = {
                            qc: ps_ot.tile(
                                [HD + 1, QC], f32,
                                tag=f"ot{qc % 2}", name="ps_ot",
                            )
                            for qc in qcs
                        }
                        for kt in range(8 * (pr + 1)):
                            jd = kt // 4
                            off = (kt % 4) * P
                            live = [qc for qc in qcs if qc >= jd]
                            ps_g = ps_s.tile(
                                [P, 2 * QC], f32, tag="ps_s", name="ps_g"
                            )
                            er_g = expr.tile(
                                [P, 2 * QC], bf16, tag="er", name="er_g"
                            )
                            segs = []
                            for qc in live:
                                diag = qc == jd
                                o0 = off if diag else 0
                                c0 = (qc - qcs[0]) * QC + o0
                                w = QC - o0
                                nc.tensor.matmul(
                                    ps_g[:, c0 : c0 + w],
                                    kTr[:, hm, kt * P : (kt + 1) * P],
                                    qTr[:, h, qc * QC + o0 : (qc + 1) * QC],
                                    start=True,
                                    stop=not diag,
                                )
                                if diag:
                                    # causal bias: += ident.T @ utri
                                    # (-200 above the diagonal; exp -> 0)
                                    nc.tensor.matmul(
                                        ps_g[:, c0 : c0 + P],
                                        ident_sb[:],
                                        utri_sb[:],
                                        start=False,
                                        stop=True,
                                    )
                                segs.append((qc, c0, o0, w))
                            g0 = segs[0][1]
                            g1 = segs[-1][1] + segs[-1][3]
                            nc.scalar.activation(
                                er_g[:, g0:g1],
                                ps_g[:, g0:g1],
                                mybir.ActivationFunctionType.Exp,
                                scale=0.125,
                            )
                            pend.append((kt, h, segs, er_g, ps_ots))
                            if len(pend) >= PEND:
                                flush_one()
                while pend:
                    flush_one()

                # ---- tail: o_proj for pr1's seq tiles (ACT is idle now,
                # so split the PSUM->SBUF copies between DVE and ACT)
                for st in range(8, N_KT):
                    o_chunk(st, ("v", "s") if st % 2 == 0 else ("s", "v"))

    nc.compile()
    return nc


def _make_utri():
    # utri[p, j] = -200 where p > j: biases the 128 diagonal query cols of
    # a diagonal key tile. After the 0.125 exp scale a masked logit sits at
    # <= -19 nats (exp <= 6e-9, negligible vs row sums >= 1) while staying
    # inside the ACT exp table's domain (huge negatives misbehave).
    p = np.arange(P)[:, None]
    j = np.arange(P)[None, :]
    return np.where(p > j, np.float32(-200.0), np.float32(0.0)).astype(
        ml_dtypes.bfloat16
    )


def make_in_maps(x, Wq, Wk, Wv, Wo):
    utri = _make_utri()
    ident = np.eye(P, dtype=np.float32).astype(ml_dtypes.bfloat16)

    def wtile(wT):  # [1024, 256] -> [128, 8, 256] (p, ko, m), bf16
        return np.ascontiguousarray(
            wT.reshape(KO, P, HSL).transpose(1, 0, 2)
        ).astype(ml_dtypes.bfloat16)

    in_maps = []
    for c in range(8):
        bi, g = c // 4, c % 4
        sl = slice(g * HSL, (g + 1) * HSL)
        # xS[sl, p, ko, s] = x[bi][sl*512+s, ko*128+p]
        xs = np.ascontiguousarray(
            x[bi].reshape(N_QC, QC, KO, P).transpose(0, 3, 2, 1)
        ).astype(ml_dtypes.bfloat16)
        in_maps.append(
            {
                "xS": xs,
                "wqT": wtile(Wq[sl, :].T),
                "wkT": wtile(Wk[sl, :].T),
                "wvT": wtile(Wv[sl, :].T),
                "woT": np.ascontiguousarray(
                    Wo[:, sl].T.reshape(2, P, DM).transpose(1, 0, 2)
                ).astype(ml_dtypes.bfloat16),
                "utri": utri,
                "ident": ident,
            }
        )
    return in_maps


def kernel(x, Wq, Wk, Wv, Wo):
    x = np.asarray(x, dtype=np.float32)
    Wq = np.asarray(Wq, dtype=np.float32)
    Wk = np.asarray(Wk, dtype=np.float32)
    Wv = np.asarray(Wv, dtype=np.float32)
    Wo = np.asarray(Wo, dtype=np.float32)
    b, s, dm = x.shape
    assert (b, s, dm) == (2, S, DM), (b, s, dm)

    if "nc" not in _CACHED:
        _CACHED["nc"] = build_program()
    nc = _CACHED["nc"]

    in_maps = make_in_maps(x, Wq, Wk, Wv, Wo)
    res = run_bass_kernel_spmd(nc, in_maps, core_ids=list(range(8)))

    out = np.zeros((2, S, DM), dtype=np.float32)
    for c in range(8):
        out[c // 4] += np.asarray(res.results[c]["out"]).astype(np.float32)
    return out
